# revision 23
# baseline (speedup 1.0000x reference)
"""Trainium2 Bass kernel for nn_DIT_11458972746143 (retrieval_knn).

B=16 batches sharded over 8 NeuronCores (2 per core). Per batch:
  1. KNN: PE computes t' = 2*x_i.x_j - |x_i|^2 - |x_j|^2 - 1e-7 + 0.1 per
     128-row block (K=5 augmented fp32 matmul). Mask (exclude d2<0.1) is one
     scalar_tensor_tensor: masked = min(t', -1e30*t'). Exact top-10 per row
     via DVE max8 / max_index / match_replace / max8 / max_index (matches
     jax.lax.top_k tie semantics).
  2. Index lists: max_index u32 outputs (k-major slots) -> f32 -> PE
     transpose -> u16 wrapped lists for gpsimd indirect_copy.
  3. Gather neighbor coords (src+tgt xyz) in stream layout, list position
     i = p*16 + T for point n = T*128 + p.
  4. PE-transpose gathered data to point-major layout.
  5. Triangle phase: 45 pairs/point, edge lengths from coordinates (matches
     reference rounding), sort3 min/max network, loss ratio, 10-smallest-of-45
     via max8 rounds, ACT sqrt + 2 Newton steps, mean, per-batch min,
     threshold loss-min < log(7/3)/30 (== 2*sigmoid(-30*(loss-min)) > 0.6).
"""

from contextlib import ExitStack

import numpy as np

import concourse.bass as bass
import concourse.tile as tile
from concourse import bacc, masks, mybir
from concourse.bass_utils import run_bass_kernel_spmd

F32 = mybir.dt.float32
U32 = mybir.dt.uint32
U16 = mybir.dt.uint16
OP = mybir.AluOpType
AX = mybir.AxisListType

N = 2048
NB = 16            # row blocks of 128
BPC = 2            # batches per core
K = 10
NPAIR = 45
BIGNEG = -1e30
C0 = float(np.float32(np.float64(0.1) - np.float64(1e-7)))
CTH = float(np.float32(np.log(np.float64(7.0) / 3.0) / 30.0))
EPS = 1e-6

_CACHE = {}


def rap(t, p_start, p_step, p_count, free_off, free_dims):
    """Raw AP over tile t: partitions [p_start::p_step] x free pattern."""
    base = t[:]
    pitch = base.ap[0][0]
    return bass.AP(
        tensor=t.tensor,
        offset=base.offset + p_start * pitch + free_off,
        ap=[[p_step * pitch, p_count]] + list(free_dims),
    )


def _build_setup(ctx, tc, pools, b, src_d, tgt_d, ident):
    nc = tc.nc
    sb = pools["sb"]

    # ---------------- Phase A: per-batch setup ----------------
    X = sb.tile([3, N], F32, tag="DAB")
    nc.sync.dma_start(X[:], src_d[b])
    ONES = sb.tile([1, N], F32, tag="G2")
    nc.gpsimd.memset(ONES[:], 1.0)  # per-batch; dies before G2 write
    ST = sb.tile([3, N], F32, tag="ETRI")
    nc.scalar.square(ST[:], X[:])
    SQR = sb.tile([1, N], F32, tag="SM")
    nc.gpsimd.tensor_reduce(SQR[:], ST[:], AX.C, OP.add)
    sq = SQR[:]

    # L4 on DVE and NSQ on Pool so they run concurrently (startup path)
    L4 = sb.tile([1, N], F32, tag="TL1")
    nc.vector.tensor_scalar(L4[:], sq, -1.0, C0, OP.mult, OP.add)
    NSQ = sb.tile([1, N], F32, tag="G1")
    nc.gpsimd.tensor_scalar(NSQ[:], sq, -1.0, None, OP.mult)

    LT = sb.tile([5, N], F32, tag=f"LT{b}")
    nc.scalar.copy(LT[0:3, :], X[:])
    nc.sync.dma_start(LT[3:4, :], ONES[:])
    nc.sync.dma_start(LT[4:5, :], L4[:])
    RHS = sb.tile([5, N], F32, tag=f"RHS{b}")
    nc.scalar.mul(RHS[0:3, :], X[:], 2.0)
    nc.sync.dma_start(RHS[3:4, :], NSQ[:])
    nc.sync.dma_start(RHS[4:5, :], ONES[:])

    # gather table: rows 16g+c, c in 0..5 = (sx, sy, sz, tx, ty, tz)
    D6 = sb.tile([128, N], F32, tag=f"D6{b}")
    nc.gpsimd.memset(D6[:], 0.0)
    for g in range(8):
        nc.sync.dma_start(D6[16 * g : 16 * g + 3, :], src_d[b])
        nc.sync.dma_start(D6[16 * g + 3 : 16 * g + 6, :], tgt_d[b])
    # self-coordinate tables, free-permuted n->i order (i = p*16 + T for
    # n = T*128 + p); built here so phase C isn't gated on these DMAs
    XPT = sb.tile([6, N], F32, tag=f"XPT{b}")
    for T in range(NB):
        nc.sync.dma_start(
            rap(XPT, 0, 1, 6, T, [[16, 128]]),
            rap(D6, 0, 1, 6, T * 128, [[1, 128]]),
        )
    return {"LT": LT, "RHS": RHS, "D6": D6, "XPT": XPT}


def _build_main(ctx, tc, pools, b, st, out_d, ident):
    """Generator: emits one scheduling step per next() so two batches can be
    interleaved in the engine queues (fills DVE stalls in phases C-E of one
    batch with phase-B work of the other)."""
    nc = tc.nc
    sb, sbk = pools["sb"], pools["sbk"]
    ps1, ps2 = pools["ps1"], pools["ps2"]
    LT, RHS, D6, XPT = st["LT"], st["RHS"], st["D6"], st["XPT"]

    # ---------------- Phase B: KNN per block ----------------
    # Per block: matmul -> PSUM; Act copies PSUM->SBUF (TS); Pool masks by
    # clamping the HIGH int16 halfword: valid t'<0 has hi <= -15769 so
    # min(hi,-384) leaves it bit-exact, while invalid t'>=0 (hi >= 0)
    # becomes 0xFE80:xxxx ~= -1e38 (finite; -128 would make NaNs, which
    # max8 sorts to the TOP), below every valid value. DVE then does
    # 8 interleaved-class max8 (256 each) instead of full-row max8 rounds,
    # and two full-row max_index for the exact indices. Ranks 9,10 land in
    # slots k=14,15 (max_index#2 searches M1[2:8]+M2[0:2] so value
    # duplicates straddling the rank-8/9 boundary resolve exactly).
    I16 = mybir.dt.int16
    IU32 = sb.tile([128, 256], U32, tag="IU32")  # slots k*16 + T
    for T in range(NB):
        pt = ps1.tile([128, N], F32, tag="knnpsum")
        for c in range(4):
            nc.tensor.matmul(
                pt[:, c * 512 : (c + 1) * 512],
                LT[:, T * 128 : (T + 1) * 128],
                RHS[:, c * 512 : (c + 1) * 512],
                start=True,
                stop=True,
            )
        TS = sbk.tile([128, N], F32, tag="TS")
        nc.scalar.copy(TS[:], pt[:])
        tsb = TS[:].bitcast(I16)
        hi = bass.AP(
            tensor=tsb.tensor,
            offset=tsb.offset + 1,
            ap=[list(tsb.ap[0])] + [[2, N]],
        )
        nc.gpsimd.tensor_scalar(hi, hi, -384, None, OP.min)

        V8 = sbk.tile([128, 64], F32, tag="V8")
        for c in range(8):
            nc.vector.max(V8[:, c * 8 : (c + 1) * 8], rap(TS, 0, 1, 128, c, [[8, 256]]))
        M12 = sbk.tile([128, 16], F32, tag="M12")
        nc.vector.max(M12[:, 0:8], V8[:])
        nc.vector.match_replace(V8[:], M12[:, 0:8], V8[:], -3e38)
        nc.vector.max(M12[:, 8:16], V8[:])
        nc.vector.max_index(rap(IU32, 0, 1, 128, T, [[16, 8]]), M12[:, 0:8], TS[:])
        nc.vector.max_index(
            rap(IU32, 0, 1, 128, 128 + T, [[16, 8]]), M12[:, 2:10], TS[:]
        )
        yield "B"

    IF32 = sb.tile([128, 256], F32, tag="IF32")
    nc.vector.tensor_copy(IF32[:], IU32[:])

    # ---------------- Phase C: index lists + gather ----------------
    IDX1 = sb.tile([128, 128], mybir.dt.int16, tag="IDX1")
    IDX2 = sb.tile([32, 128], mybir.dt.int16, tag="IDX2")
    pt1 = ps2.tile([128, 128], F32, tag="trpsum")
    nc.tensor.transpose(pt1[:], IF32[:, 0:128], ident[:])
    nc.vector.tensor_copy(IDX1[:], pt1[:])
    pt2 = ps2.tile([128, 128], F32, tag="trpsum")
    nc.tensor.transpose(pt2[:], IF32[:, 128:256], ident[:])
    # ranks 9,10 live in slots k=14,15 -> transposed rows 96..127
    nc.vector.tensor_copy(IDX2[0:32, :], pt2[96:128, :])
    yield "C"

    # chunked gathers (4 x 512 idxs) so phase D can start on chunk 0
    G1 = sb.tile([128, N], F32, tag="G1")
    G2 = sb.tile([32, N], F32, tag="G2")
    for c in range(4):
        cs = slice(c * 512, (c + 1) * 512)
        nc.gpsimd.ap_gather(
            G1[:, cs], D6[:], IDX1[:, c * 32 : (c + 1) * 32],
            channels=128, num_elems=N, d=1, num_idxs=512,
        )
        nc.gpsimd.ap_gather(
            G2[:, cs], D6[0:32, :], IDX2[:, c * 32 : (c + 1) * 32],
            channels=32, num_elems=N, d=1, num_idxs=512,
        )
        yield "C"

    # ---------------- Phase D: transpose gathered data to point layout -----
    GN = sb.tile([128, NB, K, 6], F32, tag="GN")
    XP = sb.tile([128, NB, 6], F32, tag="XP")
    for s in range(NB):
        q1 = ps2.tile([128, 128], F32, tag="trpsum")
        nc.tensor.transpose(q1[:], G1[:, s * 128 : (s + 1) * 128], ident[:])
        nc.scalar.copy(GN[:, s, 0:8, :], rap(q1, 0, 1, 128, 0, [[16, 8], [1, 6]]))
        q2 = ps2.tile([128, 40], F32, tag="trpsum")
        nc.tensor.transpose(
            q2[:, 0:32], G2[0:32, s * 128 : (s + 1) * 128], ident[0:32, 0:32]
        )
        nc.tensor.transpose(
            q2[:, 32:38], XPT[0:6, s * 128 : (s + 1) * 128], ident[0:6, 0:6]
        )
        nc.scalar.copy(GN[:, s, 8:10, :], rap(q2, 0, 1, 128, 0, [[16, 2], [1, 6]]))
        nc.scalar.copy(XP[:, s, :], rap(q2, 0, 1, 128, 32, [[1, 6]]))
        if s % 4 == 3:
            yield "D"

    # ---------------- Phase E: triangles + loss ----------------
    DK = sb.tile([128, NB, K, 6], F32, tag="DK")
    xp_b = rap(XP, 0, 1, 128, 0, [[6, NB], [0, K], [1, 6]])
    nc.vector.tensor_tensor(DK[:], xp_b, GN[:], OP.subtract)
    nc.scalar.square(DK[:], DK[:])
    EK = sb.tile([128, NB, K, 2], F32, tag="EK")
    nc.vector.tensor_reduce(
        EK[:], DK[:].rearrange("p s k (t c) -> p (s k t) c", c=3), AX.X, OP.add
    )
    yield "E"

    # ETRI[p, s, j, st, e]; e = (d01, d12, d02), st = (src, tgt)
    ETRI = sb.tile([128, NB, NPAIR, 2, 3], F32, tag="ETRI")
    joff = 0
    for a in range(K - 1):
        nrep = K - 1 - a
        nc.scalar.copy(
            rap(ETRI, 0, 1, 128, joff * 6 + 0, [[NPAIR * 6, NB], [6, nrep], [3, 2]]),
            rap(EK, 0, 1, 128, a * 2, [[2 * K, NB], [0, nrep], [1, 2]]),
        )
        nc.scalar.copy(
            rap(ETRI, 0, 1, 128, joff * 6 + 2, [[NPAIR * 6, NB], [6, nrep], [3, 2]]),
            rap(EK, 0, 1, 128, (a + 1) * 2, [[2 * K, NB], [2, nrep], [1, 2]]),
        )
        joff += nrep
    yield "E"

    DAB = sb.tile([128, NB, NPAIR, 6], F32, tag="DAB")
    joff = 0
    for a in range(K - 1):
        nrep = K - 1 - a
        nc.vector.tensor_tensor(
            rap(DAB, 0, 1, 128, joff * 6, [[NPAIR * 6, NB], [6, nrep], [1, 6]]),
            rap(GN, 0, 1, 128, a * 6, [[K * 6, NB], [0, nrep], [1, 6]]),
            rap(GN, 0, 1, 128, (a + 1) * 6, [[K * 6, NB], [6, nrep], [1, 6]]),
            OP.subtract,
        )
        joff += nrep
    nc.scalar.square(DAB[:], DAB[:])
    yield "E"
    e12_dst = rap(ETRI, 0, 1, 128, 1, [[6, NB * NPAIR], [3, 2]])
    nc.vector.tensor_reduce(
        e12_dst, DAB[:].rearrange("p s j (t c) -> p (s j t) c", c=3), AX.X, OP.add
    )

    # EPS on tgt lengths
    tsl = rap(ETRI, 0, 1, 128, 3, [[6, NB * NPAIR], [1, 3]])
    CEPS = sb.tile([128, 1], F32, tag="CEPS")
    nc.gpsimd.memset(CEPS[:], EPS)
    nc.scalar.activation(tsl, tsl, mybir.ActivationFunctionType.Identity, bias=CEPS[:])
    yield "E"

    # sort3 (both st at once); planes strided by 3
    def eplane(t, e):
        return rap(t, 0, 1, 128, e, [[3, NB * NPAIR * 2]])

    SRT = sb.tile([128, NB, NPAIR, 2, 3], F32, tag="SRT")
    TL1 = sb.tile([128, NB, NPAIR, 2], F32, tag="TL1")
    TH1 = sb.tile([128, NB, NPAIR, 2], F32, tag="TH1")
    e0, e1, e2 = eplane(ETRI, 0), eplane(ETRI, 1), eplane(ETRI, 2)
    s0, s1, s2 = eplane(SRT, 0), eplane(SRT, 1), eplane(SRT, 2)
    nc.vector.tensor_tensor(TL1[:], e0, e1, OP.min)
    nc.vector.tensor_tensor(TH1[:], e0, e1, OP.max)
    yield "E"
    nc.vector.tensor_tensor(s0, TL1[:], e2, OP.min)
    nc.vector.tensor_tensor(TL1[:], TL1[:], e2, OP.max)
    yield "E"
    nc.vector.tensor_tensor(s1, TH1[:], TL1[:], OP.min)
    nc.vector.tensor_tensor(s2, TH1[:], TL1[:], OP.max)
    yield "E"

    # num/den
    S_s = rap(SRT, 0, 1, 128, 0, [[6, NB * NPAIR], [1, 3]])
    S_t = rap(SRT, 0, 1, 128, 3, [[6, NB * NPAIR], [1, 3]])
    DN = sb.tile([128, NB, NPAIR, 3], F32, tag="DN")
    SM = sb.tile([128, NB, NPAIR, 3], F32, tag="SM")
    nc.vector.tensor_tensor(DN[:], S_s, S_t, OP.subtract)
    nc.vector.tensor_tensor(SM[:], S_s, S_t, OP.add)
    nc.scalar.square(DN[:], DN[:])
    nc.scalar.square(SM[:], SM[:])
    yield "E"
    NUM = sb.tile([128, NB, NPAIR], F32, tag="NUM")
    DEN = sb.tile([128, NB, NPAIR], F32, tag="DEN")
    nc.vector.tensor_reduce(NUM[:], DN[:].rearrange("p s j c -> p (s j) c"), AX.X, OP.add)
    nc.vector.tensor_reduce(DEN[:], SM[:].rearrange("p s j c -> p (s j) c"), AX.X, OP.add)
    NEG = NUM
    nc.vector.reciprocal(DEN[:], DEN[:])
    nc.vector.scalar_tensor_tensor(NEG[:], NUM[:], -1.0, DEN[:], OP.mult, OP.mult)
    yield "E"

    # top-10 smallest of 45 per (p, s)
    LV1 = sb.tile([128, NB, 8], F32, tag="LV1")
    LV2 = sb.tile([128, NB, 8], F32, tag="LV2")
    for s in range(NB):
        nc.vector.max(LV1[:, s, :], NEG[:, s, :])
        nc.vector.match_replace(NEG[:, s, :], LV1[:, s, :], NEG[:, s, :], BIGNEG)
        nc.vector.max(LV2[:, s, :], NEG[:, s, :])
        if s % 4 == 3:
            yield "E"

    V10 = sb.tile([128, NB, 10], F32, tag="V10")
    nc.scalar.copy(V10[:, :, 0:8], LV1[:])
    nc.scalar.copy(V10[:, :, 8:10], LV2[:, :, 0:2])
    LX = sb.tile([128, NB, 10], F32, tag="LX")
    nc.gpsimd.tensor_scalar(LX[:], V10[:], -1.0, EPS, OP.mult, OP.add)
    Y = sb.tile([128, NB, 10], F32, tag="Y")
    nc.scalar.activation(Y[:], LX[:], mybir.ActivationFunctionType.Sqrt)
    Q = sb.tile([128, NB, 10], F32, tag="Q")
    for _ in range(2):
        nc.vector.reciprocal(Q[:], Y[:])
        nc.vector.tensor_tensor(Q[:], LX[:], Q[:], OP.mult)
        nc.vector.tensor_tensor(Q[:], Y[:], Q[:], OP.add)
        nc.vector.tensor_scalar(Y[:], Q[:], 0.5, None, OP.mult)
    yield "E"

    SUM10 = sb.tile([128, NB], F32, tag="SUM10")
    nc.vector.tensor_reduce(SUM10[:], Y[:], AX.X, OP.add)
    LOSS = sb.tile([128, NB], F32, tag="LOSS")
    nc.gpsimd.tensor_scalar(LOSS[:], SUM10[:], 0.1, None, OP.mult)

    # batch min
    M1 = sb.tile([128, 1], F32, tag="M1")
    nc.vector.tensor_reduce(M1[:], LOSS[:], AX.X, OP.min)
    ptm = ps2.tile([1, 128], F32, tag="trpsum")
    nc.tensor.transpose(ptm[:], M1[:], ident[:])
    MR = sb.tile([1, 128], F32, tag="MR")
    nc.vector.tensor_copy(MR[:], ptm[:])
    MC = sb.tile([1, 1], F32, tag="MC")
    nc.vector.tensor_reduce(MC[:], MR[:], AX.X, OP.min)
    MB = sb.tile([128, 1], F32, tag="MB")
    nc.gpsimd.partition_broadcast(MB[:], MC[:])

    W = sb.tile([128, NB], F32, tag="W")
    nc.vector.tensor_scalar(W[:], LOSS[:], MB[:], CTH, OP.subtract, OP.is_lt)

    # out: transpose W -> WT[s, pi], then one DMA
    # n = (pi%16)*128 + 8*s + pi//16 with pi = 16m + T
    ptw = ps2.tile([16, 128], F32, tag="trpsum")
    nc.tensor.transpose(ptw[:], W[:], ident[:])
    WT = sb.tile([16, 128], F32, tag="WT")
    nc.scalar.copy(WT[:], ptw[:])
    src_ap = rap(WT, 0, 1, 16, 0, [[16, 8], [1, 16]])
    dst_ap = bass.AP(
        tensor=out_d.tensor,
        offset=out_d[b].offset,
        ap=[[8, 16], [1, 8], [128, 16]],
    )
    nc.sync.dma_start(dst_ap, src_ap)


def build_program():
    if "nc" in _CACHE:
        return _CACHE["nc"]
    nc = bacc.Bacc(
        "TRN2",
        target_bir_lowering=False,
        debug=False,
        enable_asserts=False,
        num_devices=8,
    )
    src_d = nc.dram_tensor("src", [BPC, 3, N], F32, kind="ExternalInput").ap()
    tgt_d = nc.dram_tensor("tgt", [BPC, 3, N], F32, kind="ExternalInput").ap()
    out_d = nc.dram_tensor("out", [BPC, N], F32, kind="ExternalOutput").ap()

    with tile.TileContext(nc) as tc, ExitStack() as ctx:
        sb = ctx.enter_context(tc.tile_pool(name="sb", bufs=1))
        sbk = ctx.enter_context(tc.tile_pool(name="sbk", bufs=2))
        ps1 = ctx.enter_context(tc.tile_pool(name="ps1", bufs=1, space="PSUM"))
        ps2 = ctx.enter_context(tc.tile_pool(name="ps2", bufs=4, space="PSUM"))
        pools = {"sb": sb, "sbk": sbk, "ps1": ps1, "ps2": ps2}
        ident = sb.tile([128, 128], F32, tag="ident")
        masks.make_identity(nc, ident[:])
        # PE p-state warmup: ~3us of back-to-back dummy transposes so the
        # first real matmuls run at full clock instead of the cold p-state
        for _ in range(14):
            w = ps2.tile([128, 128], F32, tag="trpsum")
            nc.tensor.transpose(w[:], ident[:], ident[:])
        sts = [
            _build_setup(ctx, tc, pools, b, src_d, tgt_d, ident) for b in range(BPC)
        ]
        gens = [
            _build_main(ctx, tc, pools, b, sts[b], out_d, ident) for b in range(BPC)
        ]
        # emit batch 0's phase B first, then round-robin so batch 1's
        # DVE-heavy phase B fills batch 0's C-E cross-engine stalls
        for _ in range(NB):
            next(gens[0])
        alive = list(gens)
        while alive:
            for g in list(alive):
                try:
                    next(g)
                except StopIteration:
                    alive.remove(g)

    nc.compile()
    _CACHE["nc"] = nc
    return nc


def kernel(**inputs):
    src = np.ascontiguousarray(np.asarray(inputs["src"], dtype=np.float32))
    tgt = np.ascontiguousarray(np.asarray(inputs["tgt"], dtype=np.float32))
    B = src.shape[0]
    ncores = 8
    bpc = B // ncores
    nc = build_program()
    in_maps = [
        {"src": src[i * bpc : (i + 1) * bpc], "tgt": tgt[i * bpc : (i + 1) * bpc]}
        for i in range(ncores)
    ]
    res = run_bass_kernel_spmd(nc, in_maps, core_ids=list(range(ncores)))
    return np.concatenate([res.results[i]["out"] for i in range(ncores)], axis=0)



# revision 25
# speedup vs baseline: 1.0456x; 1.0456x over previous
"""Trainium2 Bass kernel for nn_DIT_11458972746143 (retrieval_knn).

B=16 batches sharded over 8 NeuronCores (2 per core). Per batch:
  1. KNN: PE computes t' = 2*x_i.x_j - |x_i|^2 - |x_j|^2 - 1e-7 + 0.1 per
     128-row block (K=5 augmented fp32 matmul). Mask (exclude d2<0.1) is one
     scalar_tensor_tensor: masked = min(t', -1e30*t'). Exact top-10 per row
     via DVE max8 / max_index / match_replace / max8 / max_index (matches
     jax.lax.top_k tie semantics).
  2. Index lists: max_index u32 outputs (k-major slots) -> f32 -> PE
     transpose -> u16 wrapped lists for gpsimd indirect_copy.
  3. Gather neighbor coords (src+tgt xyz) in stream layout, list position
     i = p*16 + T for point n = T*128 + p.
  4. PE-transpose gathered data to point-major layout.
  5. Triangle phase: 45 pairs/point, edge lengths from coordinates (matches
     reference rounding), sort3 min/max network, loss ratio, 10-smallest-of-45
     via max8 rounds, ACT sqrt + 2 Newton steps, mean, per-batch min,
     threshold loss-min < log(7/3)/30 (== 2*sigmoid(-30*(loss-min)) > 0.6).
"""

from contextlib import ExitStack

import numpy as np

import concourse.bass as bass
import concourse.tile as tile
from concourse import bacc, masks, mybir
from concourse.bass_utils import run_bass_kernel_spmd

F32 = mybir.dt.float32
U32 = mybir.dt.uint32
U16 = mybir.dt.uint16
OP = mybir.AluOpType
AX = mybir.AxisListType

N = 2048
NB = 16            # row blocks of 128
BPC = 2            # batches per core
K = 10
NPAIR = 45
BIGNEG = -1e30
C0 = float(np.float32(np.float64(0.1) - np.float64(1e-7)))
CTH = float(np.float32(np.log(np.float64(7.0) / 3.0) / 30.0))
EPS = 1e-6

_CACHE = {}


def rap(t, p_start, p_step, p_count, free_off, free_dims):
    """Raw AP over tile t: partitions [p_start::p_step] x free pattern."""
    base = t[:]
    pitch = base.ap[0][0]
    return bass.AP(
        tensor=t.tensor,
        offset=base.offset + p_start * pitch + free_off,
        ap=[[p_step * pitch, p_count]] + list(free_dims),
    )


def _build_setup(ctx, tc, pools, b, src_d, tgt_d, ident):
    nc = tc.nc
    sb = pools["sb"]

    # ---------------- Phase A: per-batch setup ----------------
    X = sb.tile([3, N], F32, tag="DAB")
    nc.sync.dma_start(X[:], src_d[b])
    ONES = sb.tile([1, N], F32, tag="G2")
    nc.gpsimd.memset(ONES[:], 1.0)  # per-batch; dies before G2 write
    ST = sb.tile([3, N], F32, tag="ETRI")
    nc.scalar.square(ST[:], X[:])
    SQR = sb.tile([1, N], F32, tag="SM")
    nc.gpsimd.tensor_reduce(SQR[:], ST[:], AX.C, OP.add)
    sq = SQR[:]

    # L4 on DVE and NSQ on Pool so they run concurrently (startup path)
    L4 = sb.tile([1, N], F32, tag="TL1")
    nc.vector.tensor_scalar(L4[:], sq, -1.0, C0, OP.mult, OP.add)
    NSQ = sb.tile([1, N], F32, tag="G1")
    nc.gpsimd.tensor_scalar(NSQ[:], sq, -1.0, None, OP.mult)

    LT = sb.tile([5, N], F32, tag=f"LT{b}")
    nc.scalar.copy(LT[0:3, :], X[:])
    nc.sync.dma_start(LT[3:4, :], ONES[:])
    nc.sync.dma_start(LT[4:5, :], L4[:])
    RHS = sb.tile([5, N], F32, tag=f"RHS{b}")
    nc.scalar.mul(RHS[0:3, :], X[:], 2.0)
    nc.sync.dma_start(RHS[3:4, :], NSQ[:])
    nc.sync.dma_start(RHS[4:5, :], ONES[:])

    # gather table: rows 16g+c, c in 0..5 = (sx, sy, sz, tx, ty, tz)
    D6 = sb.tile([128, N], F32, tag=f"D6{b}")
    nc.gpsimd.memset(D6[:], 0.0)
    for g in range(8):
        nc.sync.dma_start(D6[16 * g : 16 * g + 3, :], src_d[b])
        nc.sync.dma_start(D6[16 * g + 3 : 16 * g + 6, :], tgt_d[b])
    # self-coordinate tables, free-permuted n->i order (i = p*16 + T for
    # n = T*128 + p); built here so phase C isn't gated on these DMAs
    XPT = sb.tile([6, N], F32, tag=f"XPT{b}")
    for T in range(NB):
        nc.sync.dma_start(
            rap(XPT, 0, 1, 6, T, [[16, 128]]),
            rap(D6, 0, 1, 6, T * 128, [[1, 128]]),
        )
    return {"LT": LT, "RHS": RHS, "D6": D6, "XPT": XPT}


def _build_main(ctx, tc, pools, b, st, out_d, ident):
    """Generator: emits one scheduling step per next() so two batches can be
    interleaved in the engine queues (fills DVE stalls in phases C-E of one
    batch with phase-B work of the other)."""
    nc = tc.nc
    sb, sbk = pools["sb"], pools["sbk"]
    ps1, ps2 = pools["ps1"], pools["ps2"]
    LT, RHS, D6, XPT = st["LT"], st["RHS"], st["D6"], st["XPT"]

    # ---------------- Phase B: KNN per block ----------------
    # Per block: matmul -> PSUM; Act copies PSUM->SBUF (TS); Pool masks by
    # clamping the HIGH int16 halfword: valid t'<0 has hi <= -15769 so
    # min(hi,-384) leaves it bit-exact, while invalid t'>=0 (hi >= 0)
    # becomes 0xFE80:xxxx ~= -1e38 (finite; -128 would make NaNs, which
    # max8 sorts to the TOP), below every valid value. DVE then does
    # 8 interleaved-class max8 (256 each) instead of full-row max8 rounds,
    # and two full-row max_index for the exact indices. Ranks 9,10 land in
    # slots k=14,15 (max_index#2 searches M1[2:8]+M2[0:2] so value
    # duplicates straddling the rank-8/9 boundary resolve exactly).
    I16 = mybir.dt.int16
    IU32 = sb.tile([128, 256], U32, tag="IU32")  # slots k*16 + T
    for T in range(NB):
        pt = ps1.tile([128, N], F32, tag="knnpsum")
        for c in range(4):
            nc.tensor.matmul(
                pt[:, c * 512 : (c + 1) * 512],
                LT[:, T * 128 : (T + 1) * 128],
                RHS[:, c * 512 : (c + 1) * 512],
                start=True,
                stop=True,
            )
        TS = sbk.tile([128, N], F32, tag="TS")
        nc.scalar.copy(TS[:], pt[:])
        tsb = TS[:].bitcast(I16)
        hi = bass.AP(
            tensor=tsb.tensor,
            offset=tsb.offset + 1,
            ap=[list(tsb.ap[0])] + [[2, N]],
        )
        nc.gpsimd.tensor_scalar(hi, hi, -384, None, OP.min)

        V8 = sbk.tile([128, 64], F32, tag="V8")
        for c in range(8):
            nc.vector.max(V8[:, c * 8 : (c + 1) * 8], rap(TS, 0, 1, 128, c, [[8, 256]]))
        M12 = sbk.tile([128, 16], F32, tag="M12")
        nc.vector.max(M12[:, 0:8], V8[:])
        nc.vector.match_replace(V8[:], M12[:, 0:8], V8[:], -3e38)
        nc.vector.max(M12[:, 8:16], V8[:])
        nc.vector.max_index(rap(IU32, 0, 1, 128, T, [[16, 8]]), M12[:, 0:8], TS[:])
        nc.vector.max_index(
            rap(IU32, 0, 1, 128, 128 + T, [[16, 8]]), M12[:, 2:10], TS[:]
        )
        yield "B"

    IF32 = sb.tile([128, 256], F32, tag="IF32")
    nc.vector.tensor_copy(IF32[:], IU32[:])

    # ---------------- Phase C: index lists + gather ----------------
    IDX1 = sb.tile([128, 128], mybir.dt.int16, tag="IDX1")
    IDX2 = sb.tile([32, 128], mybir.dt.int16, tag="IDX2")
    pt1 = ps2.tile([128, 128], F32, tag="trpsum")
    nc.tensor.transpose(pt1[:], IF32[:, 0:128], ident[:])
    nc.vector.tensor_copy(IDX1[:], pt1[:])
    pt2 = ps2.tile([128, 128], F32, tag="trpsum")
    nc.tensor.transpose(pt2[:], IF32[:, 128:256], ident[:])
    # ranks 9,10 live in slots k=14,15 -> transposed rows 96..127
    nc.vector.tensor_copy(IDX2[0:32, :], pt2[96:128, :])
    yield "C"

    # gather cost is driven by table size (2048), so one call per table
    G1 = sb.tile([128, N], F32, tag="G1")
    G2 = sb.tile([32, N], F32, tag="G2")
    nc.gpsimd.ap_gather(
        G1[:], D6[:], IDX1[:], channels=128, num_elems=N, d=1, num_idxs=N
    )
    yield "C"
    nc.gpsimd.ap_gather(
        G2[:], D6[0:32, :], IDX2[:], channels=32, num_elems=N, d=1, num_idxs=N
    )
    yield "C"

    # ---------------- Phase D: transpose gathered data to point layout -----
    GN = sb.tile([128, NB, K, 6], F32, tag="GN")
    XP = sb.tile([128, NB, 6], F32, tag="XP")
    for s in range(NB):
        q1 = ps2.tile([128, 128], F32, tag="trpsum")
        nc.tensor.transpose(q1[:], G1[:, s * 128 : (s + 1) * 128], ident[:])
        nc.scalar.copy(GN[:, s, 0:8, :], rap(q1, 0, 1, 128, 0, [[16, 8], [1, 6]]))
        q2 = ps2.tile([128, 40], F32, tag="trpsum")
        nc.tensor.transpose(
            q2[:, 0:32], G2[0:32, s * 128 : (s + 1) * 128], ident[0:32, 0:32]
        )
        nc.tensor.transpose(
            q2[:, 32:38], XPT[0:6, s * 128 : (s + 1) * 128], ident[0:6, 0:6]
        )
        nc.scalar.copy(GN[:, s, 8:10, :], rap(q2, 0, 1, 128, 0, [[16, 2], [1, 6]]))
        nc.scalar.copy(XP[:, s, :], rap(q2, 0, 1, 128, 32, [[1, 6]]))
        if s % 4 == 3:
            yield "D"

    # ---------------- Phase E: triangles + loss ----------------
    DK = sb.tile([128, NB, K, 6], F32, tag="DK")
    xp_b = rap(XP, 0, 1, 128, 0, [[6, NB], [0, K], [1, 6]])
    nc.vector.tensor_tensor(DK[:], xp_b, GN[:], OP.subtract)
    nc.scalar.square(DK[:], DK[:])
    EK = sb.tile([128, NB, K, 2], F32, tag="EK")
    nc.vector.tensor_reduce(
        EK[:], DK[:].rearrange("p s k (t c) -> p (s k t) c", c=3), AX.X, OP.add
    )
    yield "E"

    # ETRI[p, s, j, st, e]; e = (d01, d12, d02), st = (src, tgt)
    ETRI = sb.tile([128, NB, NPAIR, 2, 3], F32, tag="ETRI")
    joff = 0
    for a in range(K - 1):
        nrep = K - 1 - a
        nc.scalar.copy(
            rap(ETRI, 0, 1, 128, joff * 6 + 0, [[NPAIR * 6, NB], [6, nrep], [3, 2]]),
            rap(EK, 0, 1, 128, a * 2, [[2 * K, NB], [0, nrep], [1, 2]]),
        )
        nc.scalar.copy(
            rap(ETRI, 0, 1, 128, joff * 6 + 2, [[NPAIR * 6, NB], [6, nrep], [3, 2]]),
            rap(EK, 0, 1, 128, (a + 1) * 2, [[2 * K, NB], [2, nrep], [1, 2]]),
        )
        joff += nrep
    yield "E"

    DAB = sb.tile([128, NB, NPAIR, 6], F32, tag="DAB")
    joff = 0
    for a in range(K - 1):
        nrep = K - 1 - a
        nc.vector.tensor_tensor(
            rap(DAB, 0, 1, 128, joff * 6, [[NPAIR * 6, NB], [6, nrep], [1, 6]]),
            rap(GN, 0, 1, 128, a * 6, [[K * 6, NB], [0, nrep], [1, 6]]),
            rap(GN, 0, 1, 128, (a + 1) * 6, [[K * 6, NB], [6, nrep], [1, 6]]),
            OP.subtract,
        )
        joff += nrep
    nc.scalar.square(DAB[:], DAB[:])
    yield "E"
    e12_dst = rap(ETRI, 0, 1, 128, 1, [[6, NB * NPAIR], [3, 2]])
    nc.vector.tensor_reduce(
        e12_dst, DAB[:].rearrange("p s j (t c) -> p (s j t) c", c=3), AX.X, OP.add
    )

    # EPS on tgt lengths
    tsl = rap(ETRI, 0, 1, 128, 3, [[6, NB * NPAIR], [1, 3]])
    CEPS = sb.tile([128, 1], F32, tag="CEPS")
    nc.gpsimd.memset(CEPS[:], EPS)
    nc.scalar.activation(tsl, tsl, mybir.ActivationFunctionType.Identity, bias=CEPS[:])
    yield "E"

    # sort3 (both st at once); planes strided by 3
    def eplane(t, e):
        return rap(t, 0, 1, 128, e, [[3, NB * NPAIR * 2]])

    SRT = sb.tile([128, NB, NPAIR, 2, 3], F32, tag="SRT")
    TL1 = sb.tile([128, NB, NPAIR, 2], F32, tag="TL1")
    TH1 = sb.tile([128, NB, NPAIR, 2], F32, tag="TH1")
    e0, e1, e2 = eplane(ETRI, 0), eplane(ETRI, 1), eplane(ETRI, 2)
    s0, s1, s2 = eplane(SRT, 0), eplane(SRT, 1), eplane(SRT, 2)
    nc.vector.tensor_tensor(TL1[:], e0, e1, OP.min)
    nc.vector.tensor_tensor(TH1[:], e0, e1, OP.max)
    yield "E"
    nc.vector.tensor_tensor(s0, TL1[:], e2, OP.min)
    nc.vector.tensor_tensor(TL1[:], TL1[:], e2, OP.max)
    yield "E"
    nc.vector.tensor_tensor(s1, TH1[:], TL1[:], OP.min)
    nc.vector.tensor_tensor(s2, TH1[:], TL1[:], OP.max)
    yield "E"

    # num/den
    S_s = rap(SRT, 0, 1, 128, 0, [[6, NB * NPAIR], [1, 3]])
    S_t = rap(SRT, 0, 1, 128, 3, [[6, NB * NPAIR], [1, 3]])
    DN = sb.tile([128, NB, NPAIR, 3], F32, tag="DN")
    SM = sb.tile([128, NB, NPAIR, 3], F32, tag="SM")
    nc.vector.tensor_tensor(DN[:], S_s, S_t, OP.subtract)
    nc.vector.tensor_tensor(SM[:], S_s, S_t, OP.add)
    nc.scalar.square(DN[:], DN[:])
    nc.scalar.square(SM[:], SM[:])
    yield "E"
    NUM = sb.tile([128, NB, NPAIR], F32, tag="NUM")
    DEN = sb.tile([128, NB, NPAIR], F32, tag="DEN")
    nc.vector.tensor_reduce(NUM[:], DN[:].rearrange("p s j c -> p (s j) c"), AX.X, OP.add)
    nc.vector.tensor_reduce(DEN[:], SM[:].rearrange("p s j c -> p (s j) c"), AX.X, OP.add)
    NEG = NUM
    nc.vector.reciprocal(DEN[:], DEN[:])
    nc.vector.scalar_tensor_tensor(NEG[:], NUM[:], -1.0, DEN[:], OP.mult, OP.mult)
    yield "E"

    # top-10 smallest of 45 per (p, s)
    LV1 = sb.tile([128, NB, 8], F32, tag="LV1")
    LV2 = sb.tile([128, NB, 8], F32, tag="LV2")
    for s in range(NB):
        nc.vector.max(LV1[:, s, :], NEG[:, s, :])
        nc.vector.match_replace(NEG[:, s, :], LV1[:, s, :], NEG[:, s, :], BIGNEG)
        nc.vector.max(LV2[:, s, :], NEG[:, s, :])
        if s % 4 == 3:
            yield "E"

    V10 = sb.tile([128, NB, 10], F32, tag="V10")
    nc.scalar.copy(V10[:, :, 0:8], LV1[:])
    nc.scalar.copy(V10[:, :, 8:10], LV2[:, :, 0:2])
    LX = sb.tile([128, NB, 10], F32, tag="LX")
    nc.gpsimd.tensor_scalar(LX[:], V10[:], -1.0, EPS, OP.mult, OP.add)
    Y = sb.tile([128, NB, 10], F32, tag="Y")
    nc.scalar.activation(Y[:], LX[:], mybir.ActivationFunctionType.Sqrt)
    Q = sb.tile([128, NB, 10], F32, tag="Q")
    for _ in range(2):
        nc.vector.reciprocal(Q[:], Y[:])
        nc.vector.tensor_tensor(Q[:], LX[:], Q[:], OP.mult)
        nc.vector.tensor_tensor(Q[:], Y[:], Q[:], OP.add)
        nc.vector.tensor_scalar(Y[:], Q[:], 0.5, None, OP.mult)
    yield "E"

    SUM10 = sb.tile([128, NB], F32, tag="SUM10")
    nc.vector.tensor_reduce(SUM10[:], Y[:], AX.X, OP.add)
    LOSS = sb.tile([128, NB], F32, tag="LOSS")
    nc.gpsimd.tensor_scalar(LOSS[:], SUM10[:], 0.1, None, OP.mult)

    # batch min
    M1 = sb.tile([128, 1], F32, tag="M1")
    nc.vector.tensor_reduce(M1[:], LOSS[:], AX.X, OP.min)
    ptm = ps2.tile([1, 128], F32, tag="trpsum")
    nc.tensor.transpose(ptm[:], M1[:], ident[:])
    MR = sb.tile([1, 128], F32, tag="MR")
    nc.vector.tensor_copy(MR[:], ptm[:])
    MC = sb.tile([1, 1], F32, tag="MC")
    nc.vector.tensor_reduce(MC[:], MR[:], AX.X, OP.min)
    MB = sb.tile([128, 1], F32, tag="MB")
    nc.gpsimd.partition_broadcast(MB[:], MC[:])

    W = sb.tile([128, NB], F32, tag="W")
    nc.vector.tensor_scalar(W[:], LOSS[:], MB[:], CTH, OP.subtract, OP.is_lt)

    # out: transpose W -> WT[s, pi], then one DMA
    # n = (pi%16)*128 + 8*s + pi//16 with pi = 16m + T
    ptw = ps2.tile([16, 128], F32, tag="trpsum")
    nc.tensor.transpose(ptw[:], W[:], ident[:])
    WT = sb.tile([16, 128], F32, tag="WT")
    nc.scalar.copy(WT[:], ptw[:])
    src_ap = rap(WT, 0, 1, 16, 0, [[16, 8], [1, 16]])
    dst_ap = bass.AP(
        tensor=out_d.tensor,
        offset=out_d[b].offset,
        ap=[[8, 16], [1, 8], [128, 16]],
    )
    nc.sync.dma_start(dst_ap, src_ap)


def build_program():
    if "nc" in _CACHE:
        return _CACHE["nc"]
    nc = bacc.Bacc(
        "TRN2",
        target_bir_lowering=False,
        debug=False,
        enable_asserts=False,
        num_devices=8,
    )
    src_d = nc.dram_tensor("src", [BPC, 3, N], F32, kind="ExternalInput").ap()
    tgt_d = nc.dram_tensor("tgt", [BPC, 3, N], F32, kind="ExternalInput").ap()
    out_d = nc.dram_tensor("out", [BPC, N], F32, kind="ExternalOutput").ap()

    with tile.TileContext(nc) as tc, ExitStack() as ctx:
        sb = ctx.enter_context(tc.tile_pool(name="sb", bufs=1))
        sbk = ctx.enter_context(tc.tile_pool(name="sbk", bufs=2))
        ps1 = ctx.enter_context(tc.tile_pool(name="ps1", bufs=1, space="PSUM"))
        ps2 = ctx.enter_context(tc.tile_pool(name="ps2", bufs=4, space="PSUM"))
        pools = {"sb": sb, "sbk": sbk, "ps1": ps1, "ps2": ps2}
        ident = sb.tile([128, 128], F32, tag="ident")
        masks.make_identity(nc, ident[:])
        # PE p-state warmup: ~3us of back-to-back dummy transposes so the
        # first real matmuls run at full clock instead of the cold p-state
        for _ in range(14):
            w = ps2.tile([128, 128], F32, tag="trpsum")
            nc.tensor.transpose(w[:], ident[:], ident[:])
        # Emission order matters: engine queues are in-order, so a stalled
        # instruction blocks everything behind it on that engine. Emit
        # batch 0's phase B first (with batch 1's setup tucked behind the
        # first blocks), then interleave batch 0's C-E at 2 steps per
        # batch 1 B-step so batch 0's C-E finishes emitting before batch 1
        # leaves its stall-free B phase.
        st0 = _build_setup(ctx, tc, pools, 0, src_d, tgt_d, ident)
        g0 = _build_main(ctx, tc, pools, 0, st0, out_d, ident)
        for _ in range(4):
            next(g0)
        st1 = _build_setup(ctx, tc, pools, 1, src_d, tgt_d, ident)
        g1 = _build_main(ctx, tc, pools, 1, st1, out_d, ident)
        for _ in range(NB - 4):
            next(g0)
        alive0 = True
        while alive0:
            next(g1)
            for _ in range(2):
                try:
                    next(g0)
                except StopIteration:
                    alive0 = False
                    break
        for _ in g1:
            pass

    nc.compile()
    _CACHE["nc"] = nc
    return nc


def kernel(**inputs):
    src = np.ascontiguousarray(np.asarray(inputs["src"], dtype=np.float32))
    tgt = np.ascontiguousarray(np.asarray(inputs["tgt"], dtype=np.float32))
    B = src.shape[0]
    ncores = 8
    bpc = B // ncores
    nc = build_program()
    in_maps = [
        {"src": src[i * bpc : (i + 1) * bpc], "tgt": tgt[i * bpc : (i + 1) * bpc]}
        for i in range(ncores)
    ]
    res = run_bass_kernel_spmd(nc, in_maps, core_ids=list(range(ncores)))
    return np.concatenate([res.results[i]["out"] for i in range(ncores)], axis=0)

